# revision 24
# baseline (speedup 1.0000x reference)
"""CapsuleLayer dynamic-routing kernel for 8 trn2 NeuronCores.

Sharding: route nodes (N=2048) split across 8 cores (256 each); x and W
sharded by n, weights never replicated.  Per routing iteration the only
cross-core exchange is an AllReduce of the 64x32x32 partial sum s (bf16,
split into two j-halves so squash/next-iter work starts early).  The
final iteration skips the AllReduce entirely: cores export fp32
s-partials and the host sums + squashes.

Per-core dataflow (bf16 operands, fp32 PSUM):
  warmup: 128-element AllReduce at t=0 absorbs the ~100us cross-core
          rendezvous + channel-init into the weight-DMA window
  iter0:  s0 = (1/J) sum_{n,i} x*W           K=(n,i) matmuls, W_C layout
  iter t: Wv[b,n,j,i] = sum_o W*v            K=(jq,o) matmuls, W_B layout
          xWv = Wv * x                       evac rotated Scalar/DVE/GpSimd
          a[b,n,j] = sum_i xWv               ones-blockdiag matmul, 4-pack
          c = softmax_j(blogits)             ACT exp->bf16 + DVE trees
          cx = c*x                           DVE 2x TT with broadcast APs
          s_t = sum_{n,i} cx*W               K=(n,i) matmuls, diag extract
"""

import os
import re
import sys

for _p in ("/opt/trn_rl_repo", "/root/.axon_site/_ro/trn_rl_repo"):
    if os.path.isdir(_p) and _p not in sys.path:
        sys.path.insert(0, _p)

import numpy as np
import ml_dtypes

import concourse.bass as bass
import concourse.bacc as bacc
import concourse.mybir as mybir
from concourse import tile
from concourse.vector_clock import ScopedClock
import bass_rust

BF16 = mybir.dt.bfloat16
FP32 = mybir.dt.float32
AF = mybir.ActivationFunctionType
ALU = mybir.AluOpType

N_CORES = 8
ROUTING_ITERS = 3


def _patch_tile_drain():
    """This walrus build allows only one sync wait per instruction; Tile's
    kernel-tail drain packs the whole vector clock onto one Drain.  Split it
    into one drain per outstanding proc."""

    def _drain_and_barrier(self, tick_clock, wait_clock):
        gc = tick_clock.global_clock
        ticks = eval(re.sub(r"VectorClock", "", repr(gc)))
        n = len(ticks)
        for i, v in enumerate(ticks):
            if v > 0:
                single = [0] * n
                single[i] = v
                d = self.nc.sync.drain()
                wait_clock.add_sem_waits(
                    d.ins, ScopedClock({None: bass_rust.VectorClock(single)})
                )
        self.nc.all_engine_barrier()
        popped = self.nc._tile_sem_poison_stack.pop()
        assert popped is self._sem_poison
        self.nc.clear_and_free_semaphores(list(self.sems.allocated().values()))
        self.nc.all_engine_barrier()

    tile.TileContext._drain_and_barrier = _drain_and_barrier


_patch_tile_drain()


def _dims(B, NLOC, I, J, O):
    JQ = 4
    JG = J // JQ
    PK = JQ * O            # stage-1 contraction rows
    NN = min(128 // I, NLOC)
    PC = NN * I            # (n,i) chunk partition rows
    NCH = NLOC // NN
    PN = min(128, NLOC)    # n rows per half
    NH = NLOC // PN
    CHH = NCH // NH
    assert JG * JQ == J and NN * NCH == NLOC and PN * NH == NLOC
    return JQ, JG, PK, NN, PC, NCH, PN, NH, CHH


def build_kernel(B=64, NLOC=256, I=16, J=32, O=32, n_cores=N_CORES):
    """Emit the per-core SPMD program.  Returns the Bass module."""
    JQ, JG, PK, NN, PC, NCH, PN, NH, CHH = _dims(B, NLOC, I, J, O)
    JO = J * O
    JH = JO // 2           # columns per j-half (jg 0-3 | jg 4-7)
    JGH = JG // 2
    IJO = I * J * O

    nc = bacc.Bacc("TRN2", target_bir_lowering=False, debug=False,
                   num_devices=n_cores)

    # ---- dram parameters (host-prepped layouts) ----
    wb_d = nc.declare_dram_parameter("w_b", [JG * PK, NLOC * I], BF16,
                                     isOutput=False)
    wc_d = nc.declare_dram_parameter("w_c", [NLOC, IJO], BF16, isOutput=False)
    x2_d = nc.declare_dram_parameter("x2", [NLOC, I * B], BF16, isOutput=False)
    xin_d = nc.declare_dram_parameter("x_i_n", [NCH * PC, B], BF16,
                                      isOutput=False)
    ones_d = nc.declare_dram_parameter("ones_bd", [PC, NN], BF16,
                                       isOutput=False)
    # per-core fp32 partial of s at the final routing iteration, laid out
    # [(jg-pair, jq, o), (half, b)]; host sums over cores + squashes.
    out_d = nc.declare_dram_parameter("v_out", [(JG // 2) * PK, 2 * B], FP32,
                                      isOutput=True)

    # collective bounce buffers (internal dram); one pair per (iter, j-half)
    ar_space = "Shared" if n_cores > 4 else "Local"
    ar_in = [[nc.dram_tensor(f"ar_in{t}_{hf}", [B, JH], BF16)
              for hf in range(2)] for t in range(ROUTING_ITERS - 1)]
    ar_out = [[nc.dram_tensor(f"ar_out{t}_{hf}", [B, JH], BF16,
                              addr_space=ar_space)
               for hf in range(2)] for t in range(ROUTING_ITERS - 1)]
    # tiny warm-up collective: starts the cross-core rendezvous + channel
    # init at t=0 so the first real AllReduce hits a warm path.
    warm_in = nc.dram_tensor("warm_in", [1, 128], BF16)
    warm_out = nc.dram_tensor("warm_out", [1, 128], BF16, addr_space=ar_space)
    blog_d = nc.dram_tensor("blog_spill", [NLOC, J * B], BF16)

    rg = [list(range(n_cores))]

    with tile.TileContext(nc) as tc:
        with (
            tc.tile_pool(name="wpool", bufs=1) as wpool,
            tc.tile_pool(name="small", bufs=1) as small,
            tc.tile_pool(name="work", bufs=2) as work,
            tc.tile_pool(name="cxp", bufs=2) as cxp,
            tc.tile_pool(name="xwvp", bufs=2) as xwvp,
            tc.tile_pool(name="achk", bufs=2) as achk,
            tc.tile_pool(name="ps", bufs=2, space="PSUM") as ps,
            tc.tile_pool(name="ps_wv", bufs=2, space="PSUM") as ps_wv,
            tc.tile_pool(name="ps_a", bufs=1, space="PSUM") as ps_a,
        ):
            # ---- resident tensors ----
            wc_t = []
            for h in range(NH):
                t = wpool.tile([PN, IJO], BF16, tag=f"wc{h}")
                nc.sync.dma_start(out=t[:, :], in_=wc_d[h * PN:(h + 1) * PN, :])
                wc_t.append(t)
            x2_t = []
            for h in range(NH):
                t = wpool.tile([PN, I * B], BF16, tag=f"x2_{h}")
                nc.sync.dma_start(out=t[:, :], in_=x2_d[h * PN:(h + 1) * PN, :])
                x2_t.append(t)
            xin_t = []
            for ch in range(NCH):
                t = wpool.tile([PC, B], BF16, tag=f"xin{ch}")
                nc.sync.dma_start(out=t[:, :], in_=xin_d[ch * PC:(ch + 1) * PC, :])
                xin_t.append(t)
            ones_t = wpool.tile([PC, NN], BF16, tag="ones")
            nc.sync.dma_start(out=ones_t[:, :], in_=ones_d[:, :])
            wb_t = []
            for jg in range(JG):
                t = wpool.tile([PK, NLOC * I], BF16, tag=f"wb{jg}")
                nc.sync.dma_start(out=t[:, :], in_=wb_d[jg * PK:(jg + 1) * PK, :])
                wb_t.append(t)

            # v-blockdiag rhs tiles for stage-1 (zero background, diag
            # rewritten per iteration)
            vblk = []
            for jg in range(JG):
                t = wpool.tile([PK, JQ * B], BF16, tag=f"vblk{jg}")
                nc.vector.memset(t[:, :], 0.0)
                vblk.append(t)

            # per-j-half views of s and v
            v_bf = [small.tile([B, JH], BF16, tag=f"v_bf{hf}",
                               name=f"v_bf{hf}")
                    for hf in range(2)]
            s_sb = [small.tile([B, JH], FP32, tag=f"s_sb{hf}",
                               name=f"s_sb{hf}")
                    for hf in range(2)]

            # ---------- squash on one j-half: s_sb[hf] -> v_bf[hf] ----------
            def squash_half(hf):
                JHW = J // 2
                sq = work.tile([B, JH], FP32, tag="sq", bufs=1,
                               name=f"sq{hf}")
                nc.vector.tensor_tensor(out=sq[:, :], in0=s_sb[hf][:, :],
                                        in1=s_sb[hf][:, :], op=ALU.mult)
                norm = work.tile([B, JHW], FP32, tag="norm", bufs=2)
                nc.vector.tensor_reduce(
                    out=norm[:, :].unsqueeze(2),
                    in_=sq[:, :].rearrange("p (j o) -> p j o", o=O),
                    axis=mybir.AxisListType.X, op=ALU.add)
                np1 = work.tile([B, JHW], FP32, tag="np1", bufs=2)
                nc.vector.tensor_scalar_add(out=np1[:, :], in0=norm[:, :],
                                            scalar1=1.0)
                r1 = work.tile([B, JHW], FP32, tag="r1", bufs=2)
                nc.vector.reciprocal(out=r1[:, :], in_=np1[:, :])
                ne = work.tile([B, JHW], FP32, tag="ne", bufs=2)
                nc.vector.tensor_scalar_add(out=ne[:, :], in0=norm[:, :],
                                            scalar1=1e-8)
                sr = work.tile([B, JHW], FP32, tag="sr", bufs=2)
                nc.scalar.activation(sr[:, :], ne[:, :], AF.Sqrt)
                r2 = work.tile([B, JHW], FP32, tag="r2", bufs=2)
                nc.vector.reciprocal(out=r2[:, :], in_=sr[:, :])
                sc = work.tile([B, JHW], FP32, tag="sc", bufs=2)
                nc.vector.tensor_tensor(out=sc[:, :], in0=norm[:, :],
                                        in1=r1[:, :], op=ALU.mult)
                nc.vector.tensor_tensor(out=sc[:, :], in0=sc[:, :],
                                        in1=r2[:, :], op=ALU.mult)
                sc_b = sc[:, :].unsqueeze(2).broadcast_to((B, JHW, O))
                s3 = s_sb[hf][:, :].rearrange("p (j o) -> p j o", o=O)
                nc.vector.tensor_tensor(
                    out=v_bf[hf][:, :].rearrange("p (j o) -> p j o", o=O),
                    in0=s3, in1=sc_b, op=ALU.mult)

            # ---------- iter 0: s0, one j-half at a time ----------
            for hf in range(2):
                s0_ps = ps.tile([B, JH], FP32, tag="sps", bufs=2,
                                name=f"s0ps{hf}")
                first = True
                for h in range(NH):
                    for i in range(I):
                        nc.tensor.matmul(
                            s0_ps[:, :], x2_t[h][:, i * B:(i + 1) * B],
                            wc_t[h][:, i * JO + hf * JH: i * JO + (hf + 1) * JH],
                            start=first, stop=(h == NH - 1 and i == I - 1))
                        first = False
                s0b = work.tile([B, JH], BF16, tag="s0b", bufs=2,
                                name=f"s0b{hf}")
                nc.vector.tensor_scalar_mul(out=s0b[:, :], in0=s0_ps[:, :],
                                            scalar1=1.0 / J)
                nc.sync.dma_start(out=ar_in[0][hf][:, :], in_=s0b[:, :])
                nc.gpsimd.collective_compute(
                    "AllReduce", ALU.add, replica_groups=rg,
                    ins=[ar_in[0][hf][:, :]], outs=[ar_out[0][hf][:, :]])
            for hf in range(2):
                sT = work.tile([B, JH], BF16, tag="sT", bufs=2,
                               name=f"sT0_{hf}")
                nc.sync.dma_start(out=sT[:, :], in_=ar_out[0][hf][:, :])
                nc.vector.tensor_copy(out=s_sb[hf][:, :], in_=sT[:, :])
                squash_half(hf)

            # ---------- routing iterations ----------
            for t in range(1, ROUTING_ITERS):
                final = (t == ROUTING_ITERS - 1)
                # v -> transposed slices -> block-diag rhs tiles
                for jg in range(JG):
                    vT = work.tile([PK, B], BF16, tag="vT", bufs=2,
                                   name=f"vT{t}_{jg}")
                    nc.sync.dma_start_transpose(
                        vT[:, :],
                        v_bf[jg // JGH][:, (jg % JGH) * PK:(jg % JGH + 1) * PK])
                    for jq in range(JQ):
                        nc.vector.tensor_copy(
                            out=vblk[jg][jq * O:(jq + 1) * O,
                                         jq * B:(jq + 1) * B],
                            in_=vT[jq * O:(jq + 1) * O, :])

                # agreement logits per n-half; chunks processed in groups of
                # four: the stage-2 ones-matmuls of the four chunks share one
                # [128, 1024] psum tile via PE column groups, and one wide
                # ScalarE copy evacuates all four.
                evi = 0
                c_bf = []
                for h in range(NH):
                    asb = work.tile([PN, J * B], BF16, tag="asb", bufs=2,
                                    name=f"asb{t}_{h}")
                    for chh0 in range(0, CHH, 2):
                        xwvs = []
                        for pi in range(2):
                            ch = h * CHH + chh0 + pi
                            for qw in range(2):
                                wv_ps = ps_wv.tile(
                                    [PC, 4 * JQ * B], FP32, tag="wvps",
                                    bufs=2, name=f"wvps{t}_{ch}_{qw}")
                                for jj in range(4):
                                    jg = qw * 4 + jj
                                    nc.tensor.matmul(
                                        wv_ps[:, jj * JQ * B:(jj + 1) * JQ * B],
                                        wb_t[jg][:, ch * PC:(ch + 1) * PC],
                                        vblk[jg][:, :], start=True, stop=True)
                                # evacuate + multiply by x; rotate the work:
                                # ScalarE bf16 copy + DVE 2x multiply for
                                # most, the rest straight off PSUM on DVE or
                                # GpSimd.
                                xwv = xwvp.tile([PC, 4 * JQ * B], BF16,
                                                tag="xwv", bufs=7,
                                                name=f"xwv{t}_{ch}_{qw}")
                                xb = xin_t[ch][:, :].unsqueeze(1).unsqueeze(1) \
                                    .broadcast_to((PC, 4, JQ, B))
                                xwv4 = xwv[:, :].rearrange(
                                    "p (g q b) -> p g q b", q=JQ, b=B)
                                wv4 = wv_ps[:, :].rearrange(
                                    "p (g q b) -> p g q b", q=JQ, b=B)
                                mode = evi % 8
                                evi += 1
                                if mode in (1, 3, 5, 7):  # DVE off PSUM
                                    nc.vector.tensor_tensor(
                                        out=xwv4, in0=wv4, in1=xb, op=ALU.mult)
                                else:                    # Scalar evac + DVE 2x
                                    wvs = xwvp.tile([PC, 4 * JQ * B], BF16,
                                                    tag="wvs", bufs=3,
                                                    name=f"wvs{t}_{ch}_{qw}")
                                    nc.scalar.activation(wvs[:, :], wv_ps[:, :],
                                                         AF.Copy)
                                    nc.vector.tensor_tensor(
                                        out=xwv4,
                                        in0=wvs[:, :].rearrange(
                                            "p (g q b) -> p g q b", q=JQ, b=B),
                                        in1=xb, op=ALU.mult)
                                xwvs.append(xwv)
                        # stage-2: ones-blockdiag reduce over i; the pair's
                        # chunks land in different PE column groups
                        nw = JG * JQ * B
                        ksplit = nw // 2
                        for k in range(2):
                            a_ps = ps_a.tile([64, ksplit], FP32,
                                             tag="aps", bufs=1,
                                             name=f"aps{t}_{h}_{chh0}_{k}")
                            msplit = 512
                            for pi in range(2):
                                for m in range(ksplit // msplit):
                                    nc.tensor.matmul(
                                        a_ps[32 * pi:32 * pi + NN,
                                             m * msplit:(m + 1) * msplit],
                                        ones_t[:, :],
                                        xwvs[pi * 2 + k][:, m * msplit:
                                                         (m + 1) * msplit],
                                        start=True, stop=True,
                                        tile_position=(0, 32 * pi) if pi else None)
                            ach = achk.tile([64, ksplit], BF16,
                                            tag="ach", bufs=2,
                                            name=f"ach{t}_{h}_{chh0}_{k}")
                            nc.scalar.activation(ach[:, :], a_ps[:, :],
                                                 AF.Copy)
                            for pi in range(2):
                                chh = chh0 + pi
                                nc.sync.dma_start(
                                    out=asb[chh * NN:(chh + 1) * NN,
                                            k * ksplit:(k + 1) * ksplit],
                                    in_=ach[32 * pi:32 * pi + NN, :])
                    if t == 1:
                        nc.sync.dma_start(
                            out=blog_d[h * PN:(h + 1) * PN, :], in_=asb[:, :])
                    else:
                        # fuse the b-logit accumulation into the reload DMA
                        nc.gpsimd.dma_start(
                            out=asb[:, :], in_=blog_d[h * PN:(h + 1) * PN, :],
                            accum_op=ALU.add)
                    # softmax over j (no max subtraction; logits are O(1)):
                    # ACT exp straight to bf16, DVE trees in bf16 2x mode
                    cb = work.tile([PN, J * B], BF16, tag="cb", bufs=2,
                                   name=f"cb{t}_{h}")
                    nc.scalar.activation(cb[:, :], asb[:, :], AF.Exp)
                    den = work.tile([PN, J * B // 2], BF16, tag="den", bufs=2,
                                    name=f"den{t}_{h}")
                    c3 = cb[:, :].rearrange("p (j b) -> p j b", b=B)
                    d3 = den[:, :].rearrange("p (j b) -> p j b", b=B)
                    nc.vector.tensor_tensor(
                        out=d3[:, 0:J // 2, :], in0=c3[:, 0:J // 2, :],
                        in1=c3[:, J // 2:J, :], op=ALU.add)
                    w = J // 2
                    while w > 1:
                        hw = w // 2
                        nc.vector.tensor_tensor(
                            out=d3[:, 0:hw, :], in0=d3[:, 0:hw, :],
                            in1=d3[:, hw:w, :], op=ALU.add)
                        w = hw
                    re_ = work.tile([PN, B], BF16, tag="re", bufs=2,
                                    name=f"re{t}_{h}")
                    with nc.allow_low_precision(
                            reason="softmax normalizer; error washes out "
                                   "over the n-sum"):
                        nc.vector.reciprocal(
                            out=re_[:, :], in_=d3[:, 0:1, :].squeeze(1))
                    nc.vector.tensor_tensor(
                        out=c3, in0=c3,
                        in1=re_[:, :].unsqueeze(1).broadcast_to((PN, J, B)),
                        op=ALU.mult)
                    c_bf.append(cb)

                # s_t = sum_{n,i} cx * W   (cx built in i-halves, build work
                # split between VectorE and GpSimd)
                IH = I // 2 if I % 2 == 0 else I
                cxi = 0
                for jg in range(JG):
                    s_ps = ps.tile([PK, JQ * B], FP32, tag="sps", bufs=2,
                                   name=f"sps{t}_{jg}")
                    for h in range(NH):
                        for ih in range(I // IH):
                            cx = cxp.tile([PN, IH * JQ * B], BF16, tag="cx",
                                          bufs=2, name=f"cx{t}_{jg}_{h}_{ih}")
                            i0 = ih * IH
                            c_ap = c_bf[h][:, :].rearrange(
                                "p (j b) -> p j b", b=B)[:, jg * JQ:(jg + 1) * JQ, :]
                            c_ap = c_ap.unsqueeze(1).broadcast_to(
                                (PN, IH, JQ, B))
                            x_ap = x2_t[h][:, i0 * B:(i0 + IH) * B].rearrange(
                                "p (i b) -> p i b", b=B).unsqueeze(2).broadcast_to(
                                (PN, IH, JQ, B))
                            eng = nc.gpsimd if cxi % 3 == 2 else nc.vector
                            cxi += 1
                            eng.tensor_tensor(
                                out=cx[:, :].rearrange(
                                    "p (i q b) -> p i q b", q=JQ, b=B),
                                in0=c_ap, in1=x_ap, op=ALU.mult)
                            for ii in range(IH):
                                i = i0 + ii
                                nc.tensor.matmul(
                                    s_ps[:, :],
                                    wc_t[h][:, i * JO + jg * PK: i * JO + jg * PK + PK],
                                    cx[:, ii * JQ * B:(ii + 1) * JQ * B],
                                    start=(h == 0 and i == 0),
                                    stop=(h == NH - 1 and i == I - 1))
                    if final:
                        # last iteration: export the fp32 diagonal partial
                        # straight to dram; host sums over cores + squashes.
                        if jg % 2 == 0:
                            sdf = work.tile([PK, 2 * B], FP32, tag="sdf",
                                            bufs=2, name=f"sdf{t}_{jg}")
                        half = (jg % 2) * B
                        for jq in range(JQ):
                            nc.vector.tensor_copy(
                                out=sdf[jq * O:(jq + 1) * O, half:half + B],
                                in_=s_ps[jq * O:(jq + 1) * O,
                                         jq * B:(jq + 1) * B])
                        if jg % 2 == 1:
                            pr = jg // 2
                            nc.sync.dma_start(
                                out=out_d[pr * PK:(pr + 1) * PK, :],
                                in_=sdf[:, :])
                        continue
                    # extract diagonal blocks into a jg-pair tile, then
                    # xbar-transpose to [B, (j,o)] for the AllReduce bounce
                    if jg % 2 == 0:
                        sdp = work.tile([PK, 2 * B], BF16, tag="sd", bufs=2,
                                        name=f"sd{t}_{jg}")
                    half = (jg % 2) * B
                    for jq in range(JQ):
                        nc.vector.tensor_copy(
                            out=sdp[jq * O:(jq + 1) * O, half:half + B],
                            in_=s_ps[jq * O:(jq + 1) * O,
                                     jq * B:(jq + 1) * B])
                    if jg % 2 == 1:
                        sdT = work.tile([2 * B, PK], BF16, tag="sdT", bufs=2,
                                        name=f"sdT{t}_{jg}")
                        nc.sync.dma_start_transpose(sdT[:, :], sdp[:, :])
                        hf = jg // JGH
                        for g2 in range(2):
                            jgl = (jg - 1 + g2) % JGH
                            nc.sync.dma_start(
                                out=ar_in[t][hf][:, jgl * PK:(jgl + 1) * PK],
                                in_=sdT[g2 * B:(g2 + 1) * B, :])
                    # trigger the j-half AllReduce as soon as its last pair
                    # of diagonal blocks is written out
                    if jg % JGH == JGH - 1:
                        hf = jg // JGH
                        nc.gpsimd.collective_compute(
                            "AllReduce", ALU.add, replica_groups=rg,
                            ins=[ar_in[t][hf][:, :]],
                            outs=[ar_out[t][hf][:, :]])
                if not final:
                    for hf in range(2):
                        sT = work.tile([B, JH], BF16, tag="sT", bufs=2,
                                       name=f"sT{t}_{hf}")
                        nc.sync.dma_start(out=sT[:, :], in_=ar_out[t][hf][:, :])
                        nc.vector.tensor_copy(out=s_sb[hf][:, :], in_=sT[:, :])
                        squash_half(hf)

    nc.compile()
    return nc


def prep_inputs(x, weights, n_cores=N_CORES):
    """Shard + lay out the inputs for each core."""
    B, N, I = x.shape
    _, J, O, _ = weights.shape
    NLOC = N // n_cores
    JQ, JG, PK, NN, PC, NCH, PN, NH, CHH = _dims(B, NLOC, I, J, O)
    bf = ml_dtypes.bfloat16
    ones = np.zeros((PC, NN), dtype=bf)
    for nn in range(NN):
        ones[nn * I:(nn + 1) * I, nn] = 1.0
    x = np.asarray(x, dtype=np.float32)
    weights = np.asarray(weights, dtype=np.float32)
    in_maps = []
    for r in range(n_cores):
        n0 = r * NLOC
        Wr = weights[n0:n0 + NLOC]              # [NLOC, J, O, I]
        xr = x[:, n0:n0 + NLOC, :]              # [B, NLOC, I]
        w_b = Wr.reshape(NLOC, JG, JQ, O, I).transpose(1, 2, 3, 0, 4) \
            .reshape(JG * JQ * O, NLOC * I).astype(bf)
        w_c = Wr.transpose(0, 3, 1, 2).reshape(NLOC, I * J * O).astype(bf)
        x_nib = xr.transpose(1, 2, 0)           # [NLOC, I, B]
        x2 = x_nib.reshape(NLOC, I * B).astype(bf)
        x_i_n = x_nib.reshape(NCH * PC, B).astype(bf)
        in_maps.append({
            "w_b": np.ascontiguousarray(w_b),
            "w_c": np.ascontiguousarray(w_c),
            "x2": np.ascontiguousarray(x2),
            "x_i_n": np.ascontiguousarray(x_i_n),
            "ones_bd": ones,
        })
    return in_maps


_CACHE = {}


def kernel(x, weights):
    from concourse.bass_utils import run_bass_kernel_spmd
    x = np.asarray(x)
    weights = np.asarray(weights)
    B, N, I = x.shape
    _, J, O, _ = weights.shape
    NLOC = N // N_CORES
    key = (B, N, I, J, O)
    if key not in _CACHE:
        _CACHE[key] = build_kernel(B=B, NLOC=NLOC, I=I, J=J, O=O)
    nc = _CACHE[key]
    in_maps = prep_inputs(x, weights)
    res = run_bass_kernel_spmd(nc, in_maps, list(range(N_CORES)))
    return finish_output(res.results, B, J, O)


def finish_output(results, B, J, O):
    """Sum per-core fp32 s-partials ([(jg-pair, jq, o), (half, b)] layout)
    and apply the final squash on the host."""
    JQ = 4
    acc = np.zeros(((J // (2 * JQ)) * JQ * O, 2 * B), dtype=np.float64)
    for r in results:
        acc += np.asarray(r["v_out"], dtype=np.float64)
    # [p, jq, o, half, b] -> s[b, j, o] with j = (2p + half)*JQ + jq
    s = acc.reshape(J // (2 * JQ), JQ, O, 2, B)
    s = s.transpose(4, 0, 3, 1, 2).reshape(B, J, O)
    sq_norm = np.sum(s * s, axis=-1, keepdims=True)
    v = (sq_norm / (1.0 + sq_norm)) * s / np.sqrt(sq_norm + 1e-8)
    return v.astype(np.float32)
